# revision 33
# baseline (speedup 1.0000x reference)
"""Trainium2 Bass kernel for a Griffin-style ChimeraBlock:
   pre-norm RG-LRU recurrence branch + pre-norm SwiGLU FFN, B=2, T=2048,
   D=H=2048, FFN=5632, fp32 I/O.

Parallelization over 8 NeuronCores (tensor-parallel):
  - recurrence hidden dim H sharded 8x (256 per core); scan is elementwise
    per channel -> shards cleanly (native DVE tensor_tensor_scan op).
  - rmsnorm stats travel as tiny AllReduces whose per-token scale is
    applied post-matmul (a per-column scale commutes through the
    contraction), so the collectives stay off the tensor-engine's
    critical path.
  - AllGather of raw hs (bf16) -> rec_out computed column-parallel.
  - FFN hidden sharded 8x (704 -> padded 768 per core); down-proj partials
    ReduceScattered over D; each core emits its D-shard of the final output.

The collective queue executes in order and each op costs ~25us on this
part (mostly rendezvous), so the schedule keeps the op count minimal
(17 total) and hand-orders them so every collective completes just
before its first consumer.  DMA issue is spread across engine queues:
weights on GpSimd, plain input streams on Sync, loads that wait on a
collective on Scalar (which consumes them next anyway), and all stores
on Vector (their producer), so no queue head-of-line blocks another
stream.  Matmuls run in bf16 (fp32 accumulation in PSUM); norms/scan
state in fp32; residual path keeps x in fp32.
"""

import sys

sys.path.insert(0, "/opt/trn_rl_repo")

import numpy as np
import ml_dtypes

import concourse.bass as bass
import concourse.mybir as mybir
import concourse.tile as tile
from concourse import bacc
from concourse.bass_utils import run_bass_kernel_spmd

BF16 = mybir.dt.bfloat16
F32 = mybir.dt.float32
AF = mybir.ActivationFunctionType
OP = mybir.AluOpType

B, T, D = 2, 2048, 2048
H, FFN = 2048, 5632
NC = 8
HS = H // NC          # 256 hidden shard
DS = D // NC          # 256 d-model shard (output sharding)
FS = FFN // NC        # 704 ffn shard
FSP = 768             # ffn shard padded to a multiple of 128 (pad weights = 0)
BT = B * T            # 4096
CH = 512              # time-chunk (columns)
NCH = BT // CH        # 8 chunks
CPB = T // CH         # 4 chunks per batch element (scan resets at b boundary)
NG = NCH // 2         # 4 two-chunk groups (hs AllGather granularity)
NH = NCH // 4         # 2 four-chunk halves (stats ARs + h2 AllGather)
RSG = [(0, 1), (2, 3), (4, 5), (6,), (7,)]  # ReduceScatter groups
KD = D // 128         # 16 k-tiles when contracting over D
KH = H // 128         # 16 k-tiles when contracting over H
KF = FSP // 128       # 6 k-tiles when contracting over ffn shard
EPS = 1e-6
CCONST = 8.0

NP_BF16 = ml_dtypes.bfloat16


def _r128(ap):
    # [R, N] dram view -> [128, R//128, N] (partition, k-tile, col)
    return ap.rearrange("(k p) n -> p k n", p=128)


def build_nc(phases=7, repeat=1):
    nc = bacc.Bacc("TRN2", target_bir_lowering=False, debug=False, num_devices=NC)
    rg = [list(range(NC))]

    # ---------------- kernel I/O (per core) ----------------
    xt = nc.dram_tensor("xt", [D, BT], BF16, kind="ExternalInput")      # x^T replicated
    xf32 = nc.dram_tensor("xf32", [DS, BT], F32, kind="ExternalInput")  # f32 x^T d-shard
    w3 = nc.dram_tensor("w3", [D, 3 * HS], BF16, kind="ExternalInput")  # in|ig|rg lhsT shard
    wro = nc.dram_tensor("wro", [H, DS], BF16, kind="ExternalInput")    # rec_out lhsT d-shard
    wg = nc.dram_tensor("wg", [D, FSP], BF16, kind="ExternalInput")
    wu = nc.dram_tensor("wu", [D, FSP], BF16, kind="ExternalInput")
    wd = nc.dram_tensor("wd", [FSP, D], BF16, kind="ExternalInput")
    # cols: 0 = rec_lambda, 1 = ig bias, 2 = rg bias, 3 = h0
    smalls = nc.dram_tensor("smalls", [HS, 4], F32, kind="ExternalInput")
    y = nc.dram_tensor("y", [DS, BT], F32, kind="ExternalOutput")

    with tile.TileContext(nc) as tc:
        with (
            tc.tile_pool(name="sb", bufs=2) as sb,
            tc.tile_pool(name="ps", bufs=2, space="PSUM") as ps,
            tc.tile_pool(name="dr", bufs=1, space="DRAM") as dr,
        ):
            for _rep in range(repeat):
                build_body(nc, tc, sb, ps, dr, rg,
                           xt, xf32, w3, wro, wg, wu, wd, smalls, y)
    nc.compile()
    return nc


def build_body(nc, tc, sb, ps, dr, rg, xt, xf32, w3, wro, wg, wu, wd, smalls, y):
    AG = "AllGather"
    AR = "AllReduce"
    RS = "ReduceScatter"

    # ---------------- internal DRAM ----------------
    # chunks 0/1 compute norm1 stats locally (sq_loc is just a broadcast
    # bounce buffer); chunks 2..7 share one AllReduce.
    sq_loc = dr.tile([1, 2 * CH], F32, name="sq_loc")
    ar1_in = dr.tile([1, 6 * CH], F32, name="ar1_in")
    ar1_out = dr.tile([1, 6 * CH], F32, name="ar1_out", addr_space="Shared")
    ar2_in = [dr.tile([1, 4 * CH], F32, name=f"ar2_in{h}") for h in range(NH)]
    ar2_out = [dr.tile([1, 4 * CH], F32, name=f"ar2_out{h}", addr_space="Shared")
               for h in range(NH)]
    ar3_in = [dr.tile([1, 2 * CH], F32, name=f"ar3_in{g}") for g in range(NG)]
    ar3_out = [dr.tile([1, 2 * CH], F32, name=f"ar3_out{g}", addr_space="Shared")
               for g in range(NG)]
    agin_hs = [dr.tile([HS, 2 * CH], BF16, name=f"agin_hs{g}") for g in range(NG)]
    agout_hs = [dr.tile([H, 2 * CH], BF16, name=f"agout_hs{g}", addr_space="Shared")
                for g in range(NG)]
    agin_h2 = [dr.tile([DS, 2 * CH], BF16, name=f"agin_h2{g}") for g in range(NG)]
    agout_h2 = [dr.tile([D, 2 * CH], BF16, name=f"agout_h2{g}", addr_space="Shared")
                for g in range(NG)]
    ffn_part = [dr.tile([D, len(grp) * CH], BF16, name=f"ffn_part{j}")
                for j, grp in enumerate(RSG)]
    ffn_red = [dr.tile([DS, len(grp) * CH], BF16, name=f"ffn_red{j}")
               for j, grp in enumerate(RSG)]
    xnew_dram = dr.tile([DS, BT], F32, name="xnew_dram")
    warm_in = dr.tile([1, 8], F32, name="warm_in")
    warm_out = dr.tile([1, 8], F32, name="warm_out", addr_space="Shared")

    # Everything but the weights issues from the Sync queue; emission order
    # is arranged so a DMA that waits on a collective never sits ahead of a
    # load someone needs sooner.  Weights issue from GpSimd BEFORE the first
    # collective (dma_start just enqueues; collectives block that queue).
    ldma = nc.sync.dma_start
    sdma = nc.sync.dma_start
    vdma = nc.sync.dma_start
    wdma = nc.gpsimd.dma_start

    # ---------------- constants / small tensors ----------------
    ones_bf = sb.tile([128, 1], BF16, name="ones_bf", tag="ones", bufs=1)
    nc.vector.memset(ones_bf[:], 1.0)

    def const_tile(val, cname):
        t = sb.tile([128, 1], F32, name=cname, tag=cname, bufs=1)
        nc.vector.memset(t[:], val)
        return t

    c_ln8 = const_tile(1e-8, "c_ln8")       # Ln bias
    c_eps = const_tile(EPS, "c_eps")        # rmsnorm eps
    c_1eps = const_tile(1.0 + EPS, "c_1eps")  # 1 + eps for sqrt(1 - a^2 + eps)

    smalls_sb = sb.tile([128, 2, 4], F32, name="smalls_sb", tag="smalls", bufs=1)
    wdma(out=smalls_sb[:], in_=smalls[:].rearrange("(a p) c -> p a c", p=128))
    sig_l = sb.tile([128, 2], F32, name="sig_l", tag="sig_l", bufs=1)
    nc.scalar.activation(sig_l[:], smalls_sb[:, :, 0], AF.Sigmoid)
    c8_sb = sb.tile([128, 2], F32, name="c8_sb", tag="c8", bufs=1)
    nc.scalar.activation(c8_sb[:], sig_l[:], AF.Ln, bias=c_ln8[:])
    nc.scalar.activation(c8_sb[:], c8_sb[:], AF.Copy, bias=0.0, scale=CCONST)

    def part_sumsq(cname, src_bf16_2, dst_dram_ap):
        """sum over 256 rows of src^2 ([128,2,CH] bf16) -> dram [1,CH]."""
        psq = ps.tile([1, CH], F32, name=f"psq_{cname}", tag="psq", bufs=1)
        nc.tensor.matmul(psq[:], ones_bf[:], src_bf16_2[:, 0, :],
                         start=True, stop=False)
        nc.tensor.matmul(psq[:], ones_bf[:], src_bf16_2[:, 1, :],
                         start=False, stop=True)
        sqs = sb.tile([1, CH], F32, name=f"sqs_{cname}", tag="sqs", bufs=2)
        nc.vector.tensor_copy(sqs[:], psq[:])
        vdma(out=dst_dram_ap, in_=sqs[:])

    def inv_scale(cname, src_ap, scale, width):
        """1/sqrt(mean + eps) = exp(-0.5*ln(mean+eps)) on a broadcast-DMA'd
        [128, width] tile; the partition replication rides the DMA."""
        invc = sb.tile([128, width], F32, name=f"invc_{cname}", tag="invc",
                       bufs=2)
        sdma(out=invc[:], in_=src_ap.partition_broadcast(128))
        nc.scalar.activation(invc[:], invc[:], AF.Ln, bias=c_eps[:],
                             scale=scale)
        nc.scalar.activation(invc[:], invc[:], AF.Exp, scale=-0.5)
        return invc

    # ---------------- phase 1: norm1 stats for chunks 2..7, one AR --------
    def stats1_tail():
        for c in range(2, NCH):
            q = c - 2
            cs = slice(c * CH, (c + 1) * CH)
            xft = sb.tile([128, 2, CH], F32, name=f"xft{c}", tag="xf", bufs=2)
            ldma(out=xft[:], in_=_r128(xf32[:])[:, :, cs])
            xsq = sb.tile([128, 2, CH], BF16, name=f"xsq{c}", tag="bfa", bufs=2)
            nc.vector.tensor_tensor(xsq[:], xft[:], xft[:], op=OP.mult)
            part_sumsq(f"x{c}", xsq, ar1_in[0:1, q * CH:(q + 1) * CH])
        nc.gpsimd.collective_compute(AR, OP.add, replica_groups=rg,
                                     ins=[ar1_in[:]], outs=[ar1_out[:]])

    # ---------------- weights ----------------
    w3_sb = sb.tile([128, KD, 3 * HS], BF16, name="w3_sb", tag="bigw", bufs=3)
    wro_sb = sb.tile([128, KH, DS], BF16, name="wro_sb", tag="wro", bufs=1)
    big_state = {}

    def load_w3():
        wdma(out=w3_sb[:], in_=_r128(w3[:]))
        wdma(out=wro_sb[:], in_=_r128(wro[:]))

    def load_wgu():
        wg_sb = sb.tile([128, KD, FSP], BF16, name="wg_sb", tag="bigw", bufs=3)
        wdma(out=wg_sb[:], in_=_r128(wg[:]))
        wu_sb = sb.tile([128, KD, FSP], BF16, name="wu_sb", tag="bigw", bufs=3)
        wdma(out=wu_sb[:], in_=_r128(wu[:]))
        big_state["wg"], big_state["wu"] = wg_sb, wu_sb

    def load_wd():
        wd_sb = sb.tile([128, KF, D], BF16, name="wd_sb", tag="bigw", bufs=3)
        ldma(out=wd_sb[:], in_=_r128(wd[:]))
        big_state["wd"] = wd_sb

    scan_state = {"prev": None}

    # ---------------- phase 2: in-proj + gates + scan (per chunk) ---------
    def p2_chunk(c):
        g, jj, h = c // 2, c % 2, c // 4
        cs = slice(c * CH, (c + 1) * CH)
        xc = sb.tile([128, KD, CH], BF16, name=f"xc{c}", tag="stream", bufs=2)
        ldma(out=xc[:], in_=_r128(xt[:])[:, :, cs])
        if c < 2:
            # local full-D stats from the replicated bf16 x -- no collective
            # on the critical path at kernel start (x is replicated so every
            # core computes the same sum).
            psq = ps.tile([1, CH], F32, name=f"psq_x{c}", tag="psq", bufs=1)
            for k in range(KD):
                xsqk = sb.tile([128, CH], BF16, name=f"xsq{c}_{k}", tag="xsqk",
                               bufs=4)
                nc.vector.tensor_tensor(xsqk[:], xc[:, k, :], xc[:, k, :],
                                        op=OP.mult)
                nc.tensor.matmul(psq[:], ones_bf[:], xsqk[:],
                                 start=(k == 0), stop=(k == KD - 1))
            sqs = sb.tile([1, CH], F32, name=f"sqs_x{c}", tag="sqs", bufs=2)
            nc.vector.tensor_copy(sqs[:], psq[:])
            vdma(out=sq_loc[0:1, c * CH:(c + 1) * CH], in_=sqs[:])
            src = sq_loc[0:1, c * CH:(c + 1) * CH]
        else:
            src = ar1_out[0:1, (c - 2) * CH:(c - 1) * CH]
        invc = inv_scale(f"n1_{c}", src, 1.0 / D, CH)

        zt = {}
        for p_i in range(3):  # 0: x_proj, 1: input gate, 2: recurrence gate
            for m in range(2):
                pst = ps.tile([128, CH], F32, name=f"pp{c}_{p_i}_{m}",
                              tag="mm", bufs=5)
                for k in range(KD):
                    nc.tensor.matmul(
                        pst[:],
                        w3_sb[:, k, p_i * HS + m * 128: p_i * HS + (m + 1) * 128],
                        xc[:, k, :],
                        start=(k == 0), stop=(k == KD - 1),
                    )
                z = sb.tile([128, CH], F32, name=f"z{c}_{p_i}_{m}",
                            tag=f"z{p_i}", bufs=2)
                nc.vector.tensor_tensor(z[:], pst[:], invc[:], op=OP.mult)
                zt[(p_i, m)] = z

        # gates: batch same-type activations to limit ACT table reloads
        for m in range(2):  # i_t, r_t (sigmoids, in place over z1/z2)
            nc.scalar.activation(zt[(1, m)][:], zt[(1, m)][:], AF.Sigmoid,
                                 bias=smalls_sb[:, m, 1:2])
            nc.scalar.activation(zt[(2, m)][:], zt[(2, m)][:], AF.Sigmoid,
                                 bias=smalls_sb[:, m, 2:3])
        for m in range(2):  # la = r * (C * log_a)
            nc.vector.tensor_scalar_mul(zt[(2, m)][:], zt[(2, m)][:],
                                        c8_sb[:, m:m + 1])
        for m in range(2):  # a_t (in place over z2)
            nc.scalar.activation(zt[(2, m)][:], zt[(2, m)][:], AF.Exp)
        nas = []
        for m in range(2):  # na = -a^2
            na = sb.tile([128, CH], F32, name=f"na{c}_{m}", tag="na", bufs=2)
            nc.vector.scalar_tensor_tensor(na[:], zt[(2, m)][:], -1.0,
                                           zt[(2, m)][:],
                                           op0=OP.mult, op1=OP.mult)
            nas.append(na)
        for m in range(2):  # sq = sqrt(1 + eps - a^2)
            nc.scalar.activation(nas[m][:], nas[m][:], AF.Sqrt, bias=c_1eps[:])

        hst = sb.tile([128, 2, CH], BF16, name=f"hst{c}", tag="hs", bufs=3)
        for m in range(2):
            zx = zt[(0, m)]
            nc.vector.tensor_tensor(zx[:], zt[(1, m)][:], zx[:], op=OP.mult)
            nc.vector.tensor_tensor(zx[:], nas[m][:], zx[:], op=OP.mult)
            if c % CPB == 0:
                init = smalls_sb[:, m, 3:4]
            else:
                init = scan_state["prev"][:, m, CH - 1:CH]
            nc.vector.tensor_tensor_scan(hst[:, m, :], zt[(2, m)][:], zx[:],
                                         init, op0=OP.mult, op1=OP.add)
        scan_state["prev"] = hst

        # partial sumsq of hs over the h-shard
        hsq = sb.tile([128, 2, CH], BF16, name=f"hsq{c}", tag="bfb", bufs=2)
        nc.vector.tensor_tensor(hsq[:], hst[:], hst[:], op=OP.mult)
        part_sumsq(f"h{c}", hsq, ar2_in[h][0:1, (c % 4) * CH:(c % 4 + 1) * CH])
        vdma(out=_r128(agin_hs[g][:])[:, :, jj * CH:(jj + 1) * CH], in_=hst[:])

    def ag_hs(g):
        nc.gpsimd.collective_compute(AG, OP.bypass, replica_groups=rg,
                                     ins=[agin_hs[g][:]], outs=[agout_hs[g][:]])

    def ar2_op(h):
        nc.gpsimd.collective_compute(AR, OP.add, replica_groups=rg,
                                     ins=[ar2_in[h][:]], outs=[ar2_out[h][:]])

    # ------- phase 4: rec_out + residual + norm2 stats (per 2-chunk group) -
    xnt_tiles = {}

    def p4_group(g):
        h = g // 2
        # all loads issue before the inv broadcast: the broadcast waits on
        # AR2 and must not head-of-line block the hstm/xft streams.
        hstms, xfts = {}, {}
        for c in (2 * g, 2 * g + 1):
            jj = c % 2
            cs = slice(c * CH, (c + 1) * CH)
            hstm = sb.tile([128, KH, CH], BF16, name=f"hstm{c}", tag="stream",
                           bufs=2)
            ldma(out=hstm[:], in_=_r128(agout_hs[g][:])[:, :, jj * CH:(jj + 1) * CH])
            xft = sb.tile([128, 2, CH], F32, name=f"xfr{c}", tag="xf", bufs=2)
            ldma(out=xft[:], in_=_r128(xf32[:])[:, :, cs])
            hstms[c], xfts[c] = hstm, xft
        invc = inv_scale(f"n2_{g}", ar2_out[h][0:1, (g % 2) * 2 * CH:
                                                ((g % 2) + 1) * 2 * CH],
                         1.0 / H, 2 * CH)
        for c in (2 * g, 2 * g + 1):
            jj = c % 2
            cs = slice(c * CH, (c + 1) * CH)
            hstm, xft = hstms[c], xfts[c]
            xnt = sb.tile([128, 2, CH], F32, name=f"xnt{c}", tag="xn", bufs=2)
            for m in range(2):
                pst = ps.tile([128, CH], F32, name=f"pro{c}_{m}", tag="pro",
                              bufs=2)
                for k in range(KH):
                    nc.tensor.matmul(pst[:], wro_sb[:, k, m * 128:(m + 1) * 128],
                                     hstm[:, k, :],
                                     start=(k == 0), stop=(k == KH - 1))
                ro = sb.tile([128, CH], F32, name=f"ro{c}_{m}", tag="roz",
                             bufs=2)
                nc.vector.tensor_tensor(ro[:], pst[:],
                                        invc[:, jj * CH:(jj + 1) * CH],
                                        op=OP.mult)
                nc.vector.tensor_tensor(xnt[:, m, :], ro[:], xft[:, m, :],
                                        op=OP.add)
            xnq = sb.tile([128, 2, CH], BF16, name=f"xnq{c}", tag="bfb", bufs=2)
            nc.vector.tensor_tensor(xnq[:], xnt[:], xnt[:], op=OP.mult)
            part_sumsq(f"n{c}", xnq, ar3_in[g][0:1, jj * CH:(jj + 1) * CH])
            vdma(out=_r128(xnew_dram[:])[:, :, cs], in_=xnt[:])
            xnt_tiles[c] = xnt

    def ar3_op(g):
        nc.gpsimd.collective_compute(AR, OP.add, replica_groups=rg,
                                     ins=[ar3_in[g][:]], outs=[ar3_out[g][:]])

    # ------- phase 5: h2 = rmsnorm(x_new) d-shard, AllGather per group -----
    def p5_group(g):
        invc = inv_scale(f"n3_{g}", ar3_out[g][0:1, :], 1.0 / D, 2 * CH)
        for c in (2 * g, 2 * g + 1):
            jj = c % 2
            h2t = sb.tile([128, 2, CH], BF16, name=f"h2t{c}", tag="bfa",
                          bufs=2)
            for m in range(2):
                nc.gpsimd.tensor_tensor(h2t[:, m, :], xnt_tiles[c][:, m, :],
                                        invc[:, jj * CH:(jj + 1) * CH],
                                        op=OP.mult)
            vdma(out=_r128(agin_h2[g][:])[:, :, jj * CH:(jj + 1) * CH],
                 in_=h2t[:])
        nc.gpsimd.collective_compute(AG, OP.bypass, replica_groups=rg,
                                     ins=[agin_h2[g][:]], outs=[agout_h2[g][:]])

    # ---------------- phase 6: FFN (per chunk) + ReduceScatter -------------
    def ffn_chunk(c):
        g, jj = c // 2, c % 2
        j = next(i for i, grp in enumerate(RSG) if c in grp)
        col0 = RSG[j].index(c) * CH
        h2s = sb.tile([128, KD, CH], BF16, name=f"h2s{c}", tag="stream",
                      bufs=2)
        sdma(out=h2s[:], in_=_r128(agout_h2[g][:])[:, :, jj * CH:(jj + 1) * CH])
        gu = sb.tile([128, KF, CH], BF16, name=f"gu{c}", tag="gu", bufs=2)
        wg_sb, wu_sb = big_state["wg"], big_state["wu"]
        for m in range(KF):
            psg = ps.tile([128, CH], F32, name=f"pg{c}_{m}", tag="mm", bufs=5)
            for k in range(KD):
                nc.tensor.matmul(psg[:], wg_sb[:, k, m * 128:(m + 1) * 128],
                                 h2s[:, k, :],
                                 start=(k == 0), stop=(k == KD - 1))
            gs = sb.tile([128, CH], BF16, name=f"gs{c}_{m}", tag="gsil",
                         bufs=2)
            nc.scalar.activation(gs[:], psg[:], AF.Silu)
            psu = ps.tile([128, CH], F32, name=f"pu{c}_{m}", tag="mm", bufs=5)
            for k in range(KD):
                nc.tensor.matmul(psu[:], wu_sb[:, k, m * 128:(m + 1) * 128],
                                 h2s[:, k, :],
                                 start=(k == 0), stop=(k == KD - 1))
            nc.vector.tensor_tensor(gu[:, m, :], psu[:], gs[:], op=OP.mult)
        wd_sb = big_state["wd"]
        for md in range(KD):
            psd = ps.tile([128, CH], F32, name=f"pd{c}_{md}", tag="mm", bufs=5)
            for k in range(KF):
                nc.tensor.matmul(psd[:], wd_sb[:, k, md * 128:(md + 1) * 128],
                                 gu[:, k, :],
                                 start=(k == 0), stop=(k == KF - 1))
            dst = sb.tile([128, CH], BF16, name=f"dst{c}_{md}", tag="dstage",
                          bufs=3)
            nc.vector.tensor_copy(dst[:], psd[:])
            vdma(out=ffn_part[j][md * 128:(md + 1) * 128, col0:col0 + CH],
                 in_=dst[:])

    def rs_op(j):
        nc.gpsimd.collective_compute(RS, OP.add, replica_groups=rg,
                                     ins=[ffn_part[j][:]], outs=[ffn_red[j][:]])

    # ---------------- phase 7: final residual (per RS group) ---------------
    def p7_group(j):
        for i, c in enumerate(RSG[j]):
            cs = slice(c * CH, (c + 1) * CH)
            frt = sb.tile([128, 2, CH], BF16, name=f"frt{c}", tag="bfb", bufs=2)
            sdma(out=frt[:],
                 in_=_r128(ffn_red[j][:])[:, :, i * CH:(i + 1) * CH])
            xnt3 = sb.tile([128, 2, CH], F32, name=f"xnt3_{c}", tag="xf",
                           bufs=2)
            ldma(out=xnt3[:], in_=_r128(xnew_dram[:])[:, :, cs])
            yt = sb.tile([128, 2, CH], F32, name=f"yt{c}", tag="yt", bufs=2)
            for m in range(2):
                nc.vector.tensor_tensor(yt[:, m, :], xnt3[:, m, :],
                                        frt[:, m, :], op=OP.add)
            vdma(out=_r128(y[:])[:, :, cs], in_=yt[:])

    # ---------------- pipelined emission order ----------------
    # All weight DMAs are ISSUED on GpSimd before the first collective:
    # a collective blocks the in-order GpSimd queue until it completes, but
    # a dma_start just enqueues the descriptor and moves on, so the weight
    # transfers run in the background while the collectives rendezvous.
    # The dummy warm AR absorbs the expensive first-collective init; chunks
    # 0/1 use locally-computed norm1 stats so nothing at the head waits on
    # a collective at all.
    # CC-queue order (in-order, ~25us/op):
    #   warm, AR1, AGhs0, AR2a, AGhs1, AR3g0, AGh2g0, AGhs2, AR3g1, AGh2g1,
    #   AR2b, AGhs3, AR3g2, AGh2g2, RS0, AR3g3, AGh2g3, RS1..RS4
    load_w3()
    load_wgu()
    nc.gpsimd.collective_compute(AR, OP.add, replica_groups=rg,
                                 ins=[warm_in[:]], outs=[warm_out[:]])
    p2_chunk(0)
    stats1_tail()             # AR1 (chunks 2..7)
    p2_chunk(1)
    ag_hs(0)                  # AGhs0
    p2_chunk(2)
    p2_chunk(3)
    ar2_op(0)                 # AR2a
    ag_hs(1)                  # AGhs1
    p4_group(0)
    ar3_op(0)                 # AR3g0
    p5_group(0)               # AGh2g0
    p2_chunk(4)
    p2_chunk(5)
    ag_hs(2)                  # AGhs2
    p4_group(1)
    ar3_op(1)                 # AR3g1
    p5_group(1)               # AGh2g1
    p2_chunk(6)
    p2_chunk(7)
    ar2_op(1)                 # AR2b
    ag_hs(3)                  # AGhs3
    load_wd()
    p4_group(2)
    ar3_op(2)                 # AR3g2
    p5_group(2)               # AGh2g2
    ffn_chunk(0)
    p4_group(3)
    ar3_op(3)                 # AR3g3
    p5_group(3)               # AGh2g3
    ffn_chunk(1)
    rs_op(0)
    ffn_chunk(2)
    ffn_chunk(3)
    rs_op(1)
    ffn_chunk(4)
    p7_group(0)
    ffn_chunk(5)
    rs_op(2)
    ffn_chunk(6)
    p7_group(1)
    rs_op(3)
    ffn_chunk(7)
    p7_group(2)
    rs_op(4)
    p7_group(3)
    p7_group(4)


_CACHE = {}


def _prep_inputs(inputs):
    f = np.float32
    x = np.asarray(inputs["x"], f)                       # [B, T, D]
    norm1_w = np.asarray(inputs["norm1_w"], f)
    rec_in_w = np.asarray(inputs["rec_in_w"], f)         # [H, D]
    rec_ig_w = np.asarray(inputs["rec_ig_w"], f)
    rec_ig_b = np.asarray(inputs["rec_ig_b"], f)
    rec_rg_w = np.asarray(inputs["rec_rg_w"], f)
    rec_rg_b = np.asarray(inputs["rec_rg_b"], f)
    rec_lambda = np.asarray(inputs["rec_lambda"], f)
    rec_out_w = np.asarray(inputs["rec_out_w"], f)       # [D, H]
    rec_h0 = np.asarray(inputs["rec_h0"], f)             # [1, 1, H]
    rec_norm_w = np.asarray(inputs["rec_norm_w"], f)
    norm2_w = np.asarray(inputs["norm2_w"], f)
    ffn_gate_w = np.asarray(inputs["ffn_gate_w"], f)     # [FFN, D]
    ffn_up_w = np.asarray(inputs["ffn_up_w"], f)
    ffn_down_w = np.asarray(inputs["ffn_down_w"], f)     # [D, FFN]

    xt_full = np.ascontiguousarray(
        x.reshape(BT, D).T.astype(NP_BF16))              # [D, BT]
    xt_f32 = np.ascontiguousarray(x.reshape(BT, D).T)    # [D, BT] f32

    # fold norm gains into adjacent weights; transpose into lhsT layouts
    w_in_t = (rec_in_w * norm1_w[None, :]).T             # [D, H]
    w_ig_t = (rec_ig_w * norm1_w[None, :]).T
    w_rg_t = (rec_rg_w * norm1_w[None, :]).T
    w_ro_t = (rec_out_w * rec_norm_w[None, :]).T         # [H, D]
    w_g_t = (ffn_gate_w * norm2_w[None, :]).T            # [D, FFN]
    w_u_t = (ffn_up_w * norm2_w[None, :]).T
    w_d_t = ffn_down_w.T                                 # [FFN, D]

    in_maps = []
    for r in range(NC):
        hsl = slice(r * HS, (r + 1) * HS)
        dsl = slice(r * DS, (r + 1) * DS)
        fsl = slice(r * FS, (r + 1) * FS)
        w3_r = np.concatenate(
            [w_in_t[:, hsl], w_ig_t[:, hsl], w_rg_t[:, hsl]], axis=1)
        wg_r = np.zeros((D, FSP), f)
        wg_r[:, :FS] = w_g_t[:, fsl]
        wu_r = np.zeros((D, FSP), f)
        wu_r[:, :FS] = w_u_t[:, fsl]
        wd_r = np.zeros((FSP, D), f)
        wd_r[:FS, :] = w_d_t[fsl, :]
        smalls_r = np.stack(
            [rec_lambda[hsl], rec_ig_b[hsl], rec_rg_b[hsl],
             np.broadcast_to(rec_h0[0, 0], (H,))[hsl]], axis=1)
        in_maps.append({
            "xt": xt_full,
            "xf32": np.ascontiguousarray(xt_f32[dsl, :]),
            "w3": np.ascontiguousarray(w3_r.astype(NP_BF16)),
            "wro": np.ascontiguousarray(w_ro_t[:, dsl].astype(NP_BF16)),
            "wg": np.ascontiguousarray(wg_r.astype(NP_BF16)),
            "wu": np.ascontiguousarray(wu_r.astype(NP_BF16)),
            "wd": np.ascontiguousarray(wd_r.astype(NP_BF16)),
            "smalls": np.ascontiguousarray(smalls_r.astype(f)),
        })
    return in_maps


def run_on_device(inputs, trace=False, tmpdir=None):
    if "nc" not in _CACHE:
        _CACHE["nc"] = build_nc()
    nc = _CACHE["nc"]
    in_maps = _prep_inputs(inputs)
    res = run_bass_kernel_spmd(nc, in_maps, list(range(NC)),
                               trace=trace, tmpdir=tmpdir)
    shards = [np.asarray(res.results[r]["y"]) for r in range(NC)]
    yt = np.concatenate(shards, axis=0)                  # [D, BT]
    out = np.ascontiguousarray(yt.T).reshape(B, T, D).astype(np.float32)
    return out, res


def kernel(**inputs):
    out, _ = run_on_device(inputs, trace=False)
    return out
